# revision 10
# baseline (speedup 1.0000x reference)
"""Trainium2 Bass kernel for the JobActor GNN (2-layer GIN + actor MLP + masked softmax).

Sharding: data-parallel over batch B=8 -- one graph per NeuronCore. Params replicated.

Per-core strategy (memory-bound on adj, 64 MiB fp32; DMA roofline ~187 us):
  - Stream adj from HBM exactly ONCE, casting fp32 -> bf16 during the SWDGE DMA.
  - Transpose each 128x128 block with a REGULAR matmul against a bf16 identity
    (stationary = adj block, moving = identity). Unlike transpose-mode, regular
    matmuls pipeline back-to-back (~55-80 ns warm) and keep the HAM clock warm.
  - 4 consecutive block-transposes share one [128, 512] fp32 PSUM tile; one big
    PSUM->SBUF copy (fp32 -> fp8, alternating DVE / ACT engines) evacuates them
    into the resident fp8 adjT (16 MiB, reused by both GIN layers' spmm).
  - spmm (S1): pooled.T = stationary(h natural)[j,F] x moving(adjT)[j, i-cols];
    chunk of 512 i-cols per accumulation group, interleaved with the adj stream
    (chunk c ready after slabs 4c..4c+3).
  - GIN MLPs in fp32 (identical quantization points to the reference path:
    bf16 only at feats, h1, h2, pool weights; adj fp8 is exact on {0,1,2});
    per-partition fp32 bias + ReLU fused into the ACT PSUM evacuation, which
    writes h chunks directly as bf16 so layout transposes run as bf16 matmuls.
  - Candidate gather = one-hot matmul (iota + is_equal); graph pooling rides
    column 0 of the same accumulation. Actor MLP fp32. Masked softmax on-chip.
    actor_b3 shifts all logits equally and cancels in softmax.
"""

import os
from contextlib import ExitStack

import numpy as np

import concourse.bass as bass
import concourse.bacc as bacc
import concourse.tile as tile
from concourse import mybir
from concourse.bass import ts
from concourse.bass_utils import run_bass_kernel_spmd
from concourse.masks import make_identity

B = 8
N = 4096
IN_DIM = 2
HID = 64
J = 128
P = 128          # SBUF partitions
NB = N // P      # 32 node blocks
CH = 512         # free-dim chunk for spmm / MLPs
NCH = N // CH    # 8 chunks
SLABS_PER_CH = CH // P  # 4

FP32 = mybir.dt.float32
BF16 = mybir.dt.bfloat16
FP8 = mybir.dt.float8e4
I32 = mybir.dt.int32

AF = mybir.ActivationFunctionType

LAST_EXEC_NS = None


def _build_kernel(ctx: ExitStack, tc: tile.TileContext, io: dict):
    nc = tc.nc

    consts = ctx.enter_context(tc.tile_pool(name="consts", bufs=1))
    resident = ctx.enter_context(tc.tile_pool(name="resident", bufs=1))
    nat_pool = ctx.enter_context(tc.tile_pool(name="nat", bufs=2))
    work = ctx.enter_context(tc.tile_pool(name="work", bufs=2))
    psum_tr = ctx.enter_context(tc.tile_pool(name="psum_tr", bufs=3, space="PSUM"))
    psum_acc = ctx.enter_context(tc.tile_pool(name="psum_acc", bufs=3, space="PSUM"))
    psum_sm = ctx.enter_context(tc.tile_pool(name="psum_sm", bufs=2, space="PSUM"))

    # ---------------- constants / params ----------------
    ident16 = consts.tile([P, P], BF16)
    make_identity(nc, ident16)

    # Keep the gpsimd (SWDGE) queue exclusively for the adj stream: all other
    # loads go through HWDGE (sync) in fp32, with tiny on-chip casts to bf16.
    feat_f32 = consts.tile([P, NB, IN_DIM], FP32)
    nc.sync.dma_start(out=feat_f32, in_=io["features"].rearrange("(b p) f -> p b f", p=P))
    feat_sb = consts.tile([P, NB, IN_DIM], BF16)
    nc.vector.tensor_copy(out=feat_sb, in_=feat_f32)
    pool_sb = consts.tile([P, NB], FP32)
    nc.sync.dma_start(out=pool_sb, in_=io["graph_pool"].rearrange("(b p) -> p b", p=P))

    def load_w(name, shape):
        t = consts.tile(shape, FP32, tag=name)
        nc.sync.dma_start(out=t, in_=io[name])
        return t

    # MLP weights fp32 (quarter-rate matmuls, but hidden under the adj stream
    # for layer 0 and a small share of phase C for layer 1 -- keeps the
    # quantization points identical to the fp32 reference path).
    w01 = load_w("gin0_w1", [IN_DIM, HID])
    w02 = load_w("gin0_w2", [HID, HID])
    w11 = load_w("gin1_w1", [HID, HID])
    w12 = load_w("gin1_w2", [HID, HID])
    b01 = load_w("gin0_b1", [HID, 1])
    b02 = load_w("gin0_b2", [HID, 1])
    b11 = load_w("gin1_b1", [HID, 1])
    b12 = load_w("gin1_b2", [HID, 1])
    pmi = load_w("pooled_machine_input", [HID, 1])
    aw2 = load_w("actor_w2", [HID, HID])
    ab1 = load_w("actor_b1", [HID, 1])
    ab2 = load_w("actor_b2", [HID, 1])
    aw3 = load_w("actor_w3", [HID, 1])
    aw1 = consts.tile([HID, 3, HID], FP32)
    nc.sync.dma_start(out=aw1, in_=io["actor_w1"].rearrange("(s k) m -> k s m", s=3))
    cand_sb = consts.tile([1, J], FP32)
    nc.sync.dma_start(out=cand_sb, in_=io["cand_f32"])
    mask_sb = consts.tile([1, J], FP32)
    nc.sync.dma_start(out=mask_sb, in_=io["mask_f32"])

    # Persistent activations
    adjT = resident.tile([P, NB, N], FP8)            # adj.T, resident (16 MiB)
    h1nat = resident.tile([P, NB, HID], BF16)        # h1 natural (spmm1 stationary)
    h2nat = resident.tile([P, NB, HID], BF16)        # h2 natural (readout stationary)

    adj = io["adj"]

    def gin_mlp(pXc, w_a, b_a, w_b, b_b, hnat, c):
        """2-layer fp32 ReLU MLP on transposed chunk [*, CH] + store natural h.

        hc is written bf16 straight from PSUM (same value the reference path
        would store after its bf16 hnat copy), so the 4 layout transposes run
        as fast bf16 regular matmuls."""
        psa = psum_acc.tile([HID, CH], FP32, tag="acc")
        nc.tensor.matmul(psa, w_a, pXc)
        ha = work.tile([HID, CH], FP32, tag="ha")
        nc.scalar.activation(ha, psa, AF.Relu, bias=b_a)
        psb = psum_acc.tile([HID, CH], FP32, tag="acc")
        nc.tensor.matmul(psb, w_b, ha)
        hc = work.tile([HID, CH], BF16, tag="hc")
        nc.scalar.activation(hc, psb, AF.Relu, bias=b_b)
        # -> natural layout [node, feat] via 4 small transposing matmuls
        for s in range(SLABS_PER_CH):
            pt = psum_sm.tile([P, HID], FP32, tag="pt")
            nc.tensor.matmul(pt, hc[:, ts(s, P)], ident16[:HID, :HID])
            nc.vector.tensor_copy(out=hnat[:, c * SLABS_PER_CH + s, :], in_=pt)

    # =============== pass A: stream adj once; transpose; GIN layer 0 ===============
    for ib in range(NB):
        nat = nat_pool.tile([P, N], BF16)
        nc.gpsimd.dma_start(out=nat, in_=adj[ts(ib, P), :])  # fp32 -> bf16 cast DMA
        for jq in range(NB // 4):  # 4 block-transposes -> one [128, 512] psum tile
            ptr = psum_tr.tile([P, 4, P], FP32, tag="tr")
            for k in range(4):
                jb = 4 * jq + k
                # regular matmul: out = nat_blk.T @ I  (pipelines, keeps HAM warm)
                nc.tensor.matmul(ptr[:, k, :], nat[:, ts(jb, P)], ident16)
            # one big evacuation, alternating engines: bf16 -> fp8
            dst = adjT[:, ts(jq, 4), ts(ib, P)]
            if jq % 2 == 0:
                nc.vector.tensor_copy(out=dst, in_=ptr)
            else:
                nc.scalar.copy(out=dst, in_=ptr)

        if ib % SLABS_PER_CH != SLABS_PER_CH - 1:
            continue
        c = ib // SLABS_PER_CH
        # ---- GIN layer 0 spmm for node chunk c: pooled0.T = feats.T.T @ adjT ----
        ps0 = psum_acc.tile([IN_DIM, CH], FP32, tag="acc")
        for jb in range(NB):
            nc.tensor.matmul(ps0, feat_sb[:, jb, :], adjT[:, jb, ts(c, CH)],
                             start=(jb == 0), stop=(jb == NB - 1))
        p0c = work.tile([IN_DIM, CH], FP32, tag="p0c")
        nc.scalar.copy(p0c, ps0)
        gin_mlp(p0c, w01, b01, w02, b02, h1nat, c)

    # =============== phase C: GIN layer 1 (spmm from resident adjT) ===============
    for c in range(NCH):
        ps1 = psum_acc.tile([HID, CH], FP32, tag="acc")
        for jb in range(NB):
            nc.tensor.matmul(ps1, h1nat[:, jb, :], adjT[:, jb, ts(c, CH)],
                             start=(jb == 0), stop=(jb == NB - 1))
        p1c = work.tile([HID, CH], FP32, tag="p1c")
        nc.scalar.copy(p1c, ps1)
        gin_mlp(p1c, w11, b11, w12, b12, h2nat, c)

    # =============== phase D: pooling + gather + actor MLP + masked softmax ===============
    iota_i = consts.tile([P, NB], I32)
    nc.gpsimd.iota(iota_i, pattern=[[P, NB]], base=0, channel_multiplier=1)
    iota_f = consts.tile([P, NB], FP32)
    nc.vector.tensor_copy(out=iota_f, in_=iota_i)
    ones1 = consts.tile([1, P], FP32)
    nc.vector.memset(ones1, 1.0)
    # broadcast candidate row across partitions via PE outer product
    ps_cb = psum_acc.tile([P, J], FP32, tag="acc")
    nc.tensor.matmul(ps_cb, ones1, cand_sb)
    cand_bc = consts.tile([P, J], FP32)
    nc.scalar.copy(cand_bc, ps_cb)

    # [graph_pool column | one-hot gather matrix] @ h2  -> [g | jobs.T] in one chain
    ps_gj = psum_acc.tile([HID, 1 + J], FP32, tag="acc")
    for jb in range(NB):
        rhs = work.tile([P, 1 + J], BF16, tag="rhs")
        nc.vector.tensor_copy(out=rhs[:, 0:1], in_=pool_sb[:, jb:jb + 1])
        nc.vector.tensor_scalar(
            out=rhs[:, 1:1 + J], in0=cand_bc, scalar1=iota_f[:, jb:jb + 1],
            scalar2=None, op0=mybir.AluOpType.is_equal)
        nc.tensor.matmul(ps_gj, h2nat[:, jb, :], rhs,
                         start=(jb == 0), stop=(jb == NB - 1))
    gcol = consts.tile([HID, 1], FP32)
    nc.scalar.copy(gcol, ps_gj[:, 0:1])
    jobsT = consts.tile([HID, J], FP32)
    nc.scalar.copy(jobsT, ps_gj[:, 1:1 + J])

    # combined per-partition bias: W1b.T @ g + W1c.T @ pmi + actor_b1
    ps_bc = psum_acc.tile([HID, 1], FP32, tag="acc")
    nc.tensor.matmul(ps_bc, aw1[:, 1, :], gcol, start=True, stop=False)
    nc.tensor.matmul(ps_bc, aw1[:, 2, :], pmi, start=False, stop=True)
    bias_c = consts.tile([HID, 1], FP32)
    nc.scalar.copy(bias_c, ps_bc)
    bias_tot = consts.tile([HID, 1], FP32)
    nc.vector.tensor_add(out=bias_tot, in0=bias_c, in1=ab1)

    ps_a1 = psum_acc.tile([HID, J], FP32, tag="acc")
    nc.tensor.matmul(ps_a1, aw1[:, 0, :], jobsT)
    a1 = consts.tile([HID, J], FP32)
    nc.scalar.activation(a1, ps_a1, AF.Tanh, bias=bias_tot)
    ps_a2 = psum_acc.tile([HID, J], FP32, tag="acc")
    nc.tensor.matmul(ps_a2, aw2, a1)
    a2 = consts.tile([HID, J], FP32)
    nc.scalar.activation(a2, ps_a2, AF.Tanh, bias=ab2)
    ps_s = psum_acc.tile([1, J], FP32, tag="acc")
    nc.tensor.matmul(ps_s, aw3, a2)
    scores = consts.tile([1, J], FP32)
    nc.scalar.mul(scores, ps_s, 10.0)  # actor_b3 cancels in softmax

    maskneg = consts.tile([1, J], FP32)
    nc.scalar.mul(maskneg, mask_sb, -1e30)
    smask = consts.tile([1, J], FP32)
    nc.vector.tensor_add(out=smask, in0=scores, in1=maskneg)
    mmax = consts.tile([1, 1], FP32)
    nc.vector.reduce_max(mmax, smask, axis=mybir.AxisListType.X)
    negm = consts.tile([1, 1], FP32)
    nc.scalar.mul(negm, mmax, -1.0)
    expv = consts.tile([1, J], FP32)
    nc.scalar.activation(expv, smask, AF.Exp, bias=negm)
    ssum = consts.tile([1, 1], FP32)
    nc.vector.reduce_sum(ssum, expv, axis=mybir.AxisListType.X)
    rinv = consts.tile([1, 1], FP32)
    nc.vector.reciprocal(rinv, ssum)
    probs = consts.tile([1, J], FP32)
    nc.vector.tensor_scalar_mul(probs, expv, rinv)
    nc.sync.dma_start(out=io["probs"], in_=probs)


_PARAM_SHAPES = {
    "gin0_w1": [IN_DIM, HID], "gin0_b1": [HID], "gin0_w2": [HID, HID], "gin0_b2": [HID],
    "gin1_w1": [HID, HID], "gin1_b1": [HID], "gin1_w2": [HID, HID], "gin1_b2": [HID],
    "pooled_machine_input": [HID],
    "actor_w1": [3 * HID, HID], "actor_b1": [HID],
    "actor_w2": [HID, HID], "actor_b2": [HID], "actor_w3": [HID, 1],
}

_NC_CACHE = {}


def build_nc(reps: int = 1):
    key = ("nc", reps)
    if key in _NC_CACHE:
        return _NC_CACHE[key]
    nc = bacc.Bacc("TRN2", target_bir_lowering=False, debug=False)
    io = {
        "adj": nc.dram_tensor("adj", [N, N], FP32, kind="ExternalInput").ap(),
        "features": nc.dram_tensor("features", [N, IN_DIM], FP32, kind="ExternalInput").ap(),
        "graph_pool": nc.dram_tensor("graph_pool", [N], FP32, kind="ExternalInput").ap(),
        "cand_f32": nc.dram_tensor("cand_f32", [1, J], FP32, kind="ExternalInput").ap(),
        "mask_f32": nc.dram_tensor("mask_f32", [1, J], FP32, kind="ExternalInput").ap(),
        "probs": nc.dram_tensor("probs", [1, J], FP32, kind="ExternalOutput").ap(),
    }
    for name, shape in _PARAM_SHAPES.items():
        io[name] = nc.dram_tensor(name, shape, FP32, kind="ExternalInput").ap()
    with tile.TileContext(nc) as tc:
        for _ in range(reps):
            with ExitStack() as ctx:
                _build_kernel(ctx, tc, io)
    nc.compile()  # bacc legalization: wait-splitting (1 wait/inst on TRN2), DCE, etc.
    _NC_CACHE[key] = nc
    return nc


def make_in_maps(inputs):
    in_maps = []
    for b in range(B):
        m = {
            "adj": np.ascontiguousarray(inputs["adj"][b], dtype=np.float32),
            "features": np.ascontiguousarray(inputs["features"][b], dtype=np.float32),
            "graph_pool": np.ascontiguousarray(inputs["graph_pool"][b], dtype=np.float32),
            "cand_f32": np.asarray(inputs["candidate"][b]).astype(np.float32).reshape(1, J),
            "mask_f32": np.asarray(inputs["mask"][b]).astype(np.float32).reshape(1, J),
        }
        for name in _PARAM_SHAPES:
            m[name] = np.ascontiguousarray(inputs[name], dtype=np.float32).reshape(_PARAM_SHAPES[name])
        in_maps.append(m)
    return in_maps


def kernel(**inputs) -> np.ndarray:
    global LAST_EXEC_NS
    nc = build_nc()
    in_maps = make_in_maps(inputs)
    # NTFF tracing is unavailable on this axon client (no antenv.axon_hooks);
    # always run untraced. Timing is done separately (see test.py).
    os.environ["BASS_NEVER_TRACE"] = "1"
    res = run_bass_kernel_spmd(nc, in_maps, core_ids=list(range(B)), trace=False)
    LAST_EXEC_NS = res.exec_time_ns
    out = np.stack([np.asarray(res.results[b]["probs"]).reshape(J) for b in range(B)], axis=0)
    return out.astype(np.float32)
